# revision 12
# baseline (speedup 1.0000x reference)
# Trainium2 Bass kernel for EquivariantProductBasisBlock (MACE-style product basis).
#
# Math (per node b, channel c, both output irreps l0 (d=1) / l1 (d=3)):
#   W_nu[k, c]   = sum_e y[b,e] w_nu[e,k,c]              (per-node path weights)
#   F[f, c]      = [x[c,i]*W3[k,c] (36) | W2[k,c] (3) | W1[k,c] (2)]  x2 irreps = 82
#   Y1[c, m]     = sum_f F[f,c] B[f,m]                   (one K=82 matmul, m=360)
#   E[c, m]      = Y1 * (x_p x_q | x_p broadcast)        (elementwise)
#   out[j, D]    = sum_c lin[c,j] * sum_m E[c, (D,m')]   (matmul with colliding out AP
#                                                         -> PSUM accumulates the m'-sum)
# B packs u3/u2/u1 contracted into a single [82, 360] matrix (host-side, tiny).
#
# Sharding: data-parallel over nodes, 256 nodes per core, 8 cores. U/w/lin replicated.

import numpy as np

N, C, NIRR, E = 2048, 128, 9, 10
K3, K2, K1 = 4, 3, 2
NCORES = 8
NB = N // NCORES          # nodes per core (256)
NF = 41                   # features per irrep
NFT = 2 * NF              # 82 total feature rows
MW = 360                  # 4 D-blocks x 90 (81 pq-cols + 9 p-cols)
GRP = 8                   # nodes per inner group
NGRP = NB // GRP

import os
USE_COLLISION = os.environ.get("K_COLLISION", "1") == "1"

_cache = {}


def _legalize_sync_waits(json_bytes):
    """This toolchain's walrus accepts at most ONE sync wait per instruction.
    Split extra waits onto same-engine Drain instructions inserted before."""
    import json as _json
    j = _json.loads(json_bytes)
    nid = [0]
    for f in j["functions"]:
        for blk in f["blocks"]:
            out = []
            for inst in blk["instructions"]:
                si = inst.get("sync_info") or {}
                waits = si.get("on_wait") or []
                upds = si.get("on_update") or []
                assert len(upds) <= 1, f"{inst['name']}: {len(upds)} updates"
                if len(waits) > 1:
                    for w in waits[:-1]:
                        nid[0] += 1
                        out.append({
                            "debug": inst.get("debug", 0),
                            "engine": inst["engine"],
                            "ins": [], "outs": [],
                            "name": f"LW-{nid[0]}",
                            "opcode": "Drain",
                            "sync_info": {"on_update": [], "on_wait": [w]},
                        })
                    si["on_wait"] = [waits[-1]]
                out.append(inst)
            blk["instructions"] = out
    return _json.dumps(j).encode()


def _build_program():
    import concourse.bass as bass
    import concourse.mybir as mybir
    from concourse.tile import TileContext

    fp32 = mybir.dt.float32
    nc = bass.Bass()

    xt = nc.dram_tensor("xt", [C, NB * NIRR], fp32, kind="ExternalInput")
    yt = nc.dram_tensor("yt", [E, NB], fp32, kind="ExternalInput")
    wmat = nc.dram_tensor("wmat", [E, 18 * C], fp32, kind="ExternalInput")
    bmat = nc.dram_tensor("bmat", [NFT, MW], fp32, kind="ExternalInput")
    linmat = nc.dram_tensor("linmat", [C, 2 * C], fp32, kind="ExternalInput")
    sct0 = nc.dram_tensor("sct0", [C, NB], fp32, kind="ExternalInput")
    sct1 = nc.dram_tensor("sct1", [C, 3 * NB], fp32, kind="ExternalInput")
    ident = nc.dram_tensor("ident", [C, C], fp32, kind="ExternalInput")
    outp = nc.dram_tensor("outp", [C, 4 * NB], fp32, kind="ExternalOutput")

    mult = mybir.AluOpType.mult
    add = mybir.AluOpType.add

    with TileContext(nc) as tc:
        with (
            tc.tile_pool(name="singles", bufs=1) as singles,
            tc.tile_pool(name="px", bufs=6) as px,
            tc.tile_pool(name="pxs", bufs=4) as pxs,
            tc.tile_pool(name="pxx", bufs=4) as pxx,
            tc.tile_pool(name="pxsts", bufs=3) as pxsts,
            tc.tile_pool(name="pe", bufs=10) as pe_pool,
            tc.tile_pool(name="psA", bufs=3, space="PSUM") as psA,      # y1 + setup mms
            tc.tile_pool(name="psT", bufs=2, space="PSUM") as psT,      # transposes
            tc.tile_pool(name="psO", bufs=1, space="PSUM") as psO,      # output accum
        ):
            # ---- setup: load constants ----
            identsb = singles.tile([C, C], fp32, tag="ident")
            nc.gpsimd.dma_start(identsb, ident[:, :])
            bsb = singles.tile([NFT, MW], fp32, tag="bmat")
            nc.gpsimd.dma_start(bsb, bmat[:, :])
            linsb = singles.tile([C, 2 * C], fp32, tag="linmat")
            nc.gpsimd.dma_start(linsb, linmat[:, :])
            sc0sb = singles.tile([C, NB], fp32, tag="sct0")
            nc.gpsimd.dma_start(sc0sb, sct0[:, :])
            sc1sb = singles.tile([C, 3 * NB], fp32, tag="sct1")
            nc.gpsimd.dma_start(sc1sb, sct1[:, :])
            wsb = singles.tile([E, 18 * C], fp32, tag="wmat")
            nc.gpsimd.dma_start(wsb, wmat[:, :])
            ytsb = singles.tile([E, NB], fp32, tag="yt")
            nc.gpsimd.dma_start(ytsb, yt[:, :])

            # ---- per-node path weights: W_nu[k,c] for all nodes, both irreps ----
            # wtiles[l][nu] laid out [C, k*NB + b]
            nk = [K3, K2, K1]
            wtiles = [[None] * 3 for _ in range(2)]
            si = 0
            for l in range(2):
                for nu in range(3):
                    t = singles.tile([C, nk[nu] * NB], fp32, tag=f"w_{l}_{nu}")
                    wtiles[l][nu] = t
                    for k in range(nk[nu]):
                        ps = psA.tile([C, 512], fp32, tag="y1")
                        nc.tensor.matmul(
                            ps[:, 0:NB],
                            lhsT=wsb[:, si * C:(si + 1) * C],
                            rhs=ytsb[:, :],
                        )
                        if si % 2 == 1:
                            nc.scalar.copy(t[:, k * NB:(k + 1) * NB], ps[:, 0:NB])
                        else:
                            nc.vector.tensor_copy(
                                t[:, k * NB:(k + 1) * NB], ps[:, 0:NB])
                        si += 1

            # persistent output accumulators (PSUM)
            o0ps = psO.tile([C, 512], fp32, tag="o0")
            o1psa = psO.tile([C, 512], fp32, tag="o1a")
            o1psb = psO.tile([C, 512], fp32, tag="o1b")

            fsb = None
            if not USE_COLLISION:
                fsb = singles.tile([C, 4 * NB], fp32, tag="fsb")

            # ---- main loop over groups of 8 nodes ----
            for g in range(NGRP):
                x8 = px.tile([C, GRP * NIRR], fp32, tag="x8")
                nc.sync.dma_start(x8, xt[:, g * GRP * NIRR:(g + 1) * GRP * NIRR])
                x8v = x8.rearrange("p (n i) -> p n i", i=NIRR)

                # features Xs: [C, n, 82]
                xs8 = pxs.tile([C, GRP * NFT], fp32, tag="xs8")
                xsv = xs8.rearrange("p (n f) -> p n f", f=NFT)
                for l in range(2):
                    w3v = wtiles[l][0].rearrange("p (k b) -> p b k", b=NB)
                    w3s = w3v[:, g * GRP:(g + 1) * GRP, :]
                    nc.gpsimd.tensor_tensor(
                        out=xsv[:, :, NF * l:NF * l + 36].rearrange(
                            "p n (k i) -> p n k i", i=NIRR),
                        in0=x8v.unsqueeze(2).to_broadcast([C, GRP, K3, NIRR]),
                        in1=w3s.unsqueeze(3).to_broadcast([C, GRP, K3, NIRR]),
                        op=mult,
                    )
                    w2v = wtiles[l][1].rearrange("p (k b) -> p b k", b=NB)
                    nc.gpsimd.tensor_copy(
                        xsv[:, :, NF * l + 36:NF * l + 39],
                        w2v[:, g * GRP:(g + 1) * GRP, :],
                    )
                    w1v = wtiles[l][2].rearrange("p (k b) -> p b k", b=NB)
                    nc.gpsimd.tensor_copy(
                        xsv[:, :, NF * l + 39:NF * l + 41],
                        w1v[:, g * GRP:(g + 1) * GRP, :],
                    )

                # XX: [C, n, 90] = [x_p*x_q (81) | x_p (9)]
                xx8 = pxx.tile([C, GRP * 90], fp32, tag="xx8")
                xxv = xx8.rearrange("p (n s) -> p n s", s=90)
                nc.gpsimd.tensor_tensor(
                    out=xxv[:, :, 0:81].rearrange("p n (a q) -> p n a q", q=NIRR),
                    in0=x8v.unsqueeze(3).to_broadcast([C, GRP, NIRR, NIRR]),
                    in1=x8v.unsqueeze(2).to_broadcast([C, GRP, NIRR, NIRR]),
                    op=mult,
                )
                nc.gpsimd.tensor_copy(xxv[:, :, 81:90], x8v)

                # transpose features, 4 nodes per PSUM bank
                for h in range(2):
                    tps = psT.tile([NFT, 512], fp32, tag="xsT")
                    for j in range(4):
                        nc.tensor.transpose(
                            tps[:, 128 * j:128 * (j + 1)],
                            xsv[:, 4 * h + j, :],
                            identsb[:, :],
                        )
                    tsb = pxsts.tile([NFT, 512], fp32, tag="xsTs")
                    nc.scalar.copy(tsb[:, :], tps[:, :])

                    for j in range(4):
                        node = g * GRP + 4 * h + j
                        # stage 1: Y1 = F.T @ B  -> [C, 360]
                        y1 = psA.tile([C, 512], fp32, tag="y1")
                        nc.tensor.matmul(
                            y1[:, 0:MW],
                            lhsT=tsb[:, 128 * j:128 * (j + 1)],
                            rhs=bsb[:, :],
                        )
                        # E = Y1 * XX-broadcast  [C, 4, 90]
                        esb = pe_pool.tile([C, MW], fp32, tag="esb")
                        nc.vector.tensor_tensor(
                            out=esb.rearrange("p (d s) -> p d s", s=90),
                            in0=y1[:, 0:MW].rearrange("p (d s) -> p d s", s=90),
                            in1=xxv[:, 4 * h + j, :].unsqueeze(1).to_broadcast(
                                [C, 4, 90]),
                            op=mult,
                        )
                        if USE_COLLISION:
                            # l0: out[j] += sum_s lin0[c,j] E[c,0,s]
                            nc.tensor.matmul(
                                o0ps[:, node:node + 1].to_broadcast([C, 90]),
                                lhsT=linsb[:, 0:C],
                                rhs=esb[:, 0:90],
                            )
                            op1 = o1psa if node < 128 else o1psb
                            nb3 = 3 * (node % 128)
                            nc.tensor.matmul(
                                op1[:, nb3:nb3 + 3].unsqueeze(1).to_broadcast(
                                    [C, 90, 3]),
                                lhsT=linsb[:, C:2 * C],
                                rhs=esb[:, 90:MW].rearrange(
                                    "p (d s) -> p s d", s=90),
                            )
                        else:
                            nc.vector.tensor_reduce(
                                out=fsb[:, node::NB],
                                in_=esb.rearrange("p (d s) -> p d s", s=90),
                                axis=mybir.AxisListType.X,
                                op=add,
                            )

            if not USE_COLLISION:
                # tail matmuls: O = lin.T @ F
                nc.tensor.matmul(
                    o0ps[:, 0:NB], lhsT=linsb[:, 0:C], rhs=fsb[:, 0:NB])
                f1v = fsb.rearrange("p (d b) -> p b d", d=4)[:, :, 1:4]
                nc.tensor.matmul(
                    o1psa[:, 0:384], lhsT=linsb[:, C:2 * C],
                    rhs=f1v[:, 0:128, :])
                nc.tensor.matmul(
                    o1psb[:, 0:384], lhsT=linsb[:, C:2 * C],
                    rhs=f1v[:, 128:256, :])

            # ---- add sc, store ----
            outsb = singles.tile([C, 4 * NB], fp32, tag="outsb")
            nc.vector.tensor_tensor(
                out=outsb[:, 0:NB], in0=o0ps[:, 0:NB], in1=sc0sb[:, :], op=add)
            nc.vector.tensor_tensor(
                out=outsb[:, NB:NB + 384], in0=o1psa[:, 0:384],
                in1=sc1sb[:, 0:384], op=add)
            nc.vector.tensor_tensor(
                out=outsb[:, NB + 384:4 * NB], in0=o1psb[:, 0:384],
                in1=sc1sb[:, 384:768], op=add)
            nc.sync.dma_start(outp[:, :], outsb[:, :])

    return nc


def _prep_shared(inputs):
    """Host-side tiny tensors, replicated across cores."""
    u3 = [inputs["u3_l0"], inputs["u3_l1"]]
    u2 = [inputs["u2_l0"], inputs["u2_l1"]]
    u1 = [inputs["u1_l0"], inputs["u1_l1"]]
    w3 = [inputs["w3_l0"], inputs["w3_l1"]]
    w2 = [inputs["w2_l0"], inputs["w2_l1"]]
    w1 = [inputs["w1_l0"], inputs["w1_l1"]]

    # wmat [E, 18*C]: per l: w3 k0..3, w2 k0..2, w1 k0..1, each [E, C]
    cols = []
    for l in range(2):
        for wt, nk in ((w3, K3), (w2, K2), (w1, K1)):
            for k in range(nk):
                cols.append(np.asarray(wt[l][:, k, :]))
    wmat = np.concatenate(cols, axis=1).astype(np.float32)

    # bmat [82, 360]; cols: D in {l0d0, l1d0..2}, within D: 81 (p,q) then 9 (p)
    bmat = np.zeros((NFT, MW), np.float32)
    dmap = [(0, 0), (1, 0), (1, 1), (1, 2)]
    for D, (l, d) in enumerate(dmap):
        r0 = NF * l
        u3l = np.asarray(u3[l])  # [d, 9(p), 9(q), 9(i), K3]
        u2l = np.asarray(u2[l])  # [d, 9(p), 9(i=q), K2]
        u1l = np.asarray(u1[l])  # [d, 9(p), K1]
        for k in range(K3):
            for i in range(NIRR):
                # feature row (k,i) -> col (p,q)
                bmat[r0 + k * NIRR + i, 90 * D:90 * D + 81] = u3l[d, :, :, i, k].reshape(81)
        for k in range(K2):
            bmat[r0 + 36 + k, 90 * D:90 * D + 81] = u2l[d, :, :, k].reshape(81)
        for k in range(K1):
            bmat[r0 + 39 + k, 90 * D + 81:90 * D + 90] = u1l[d, :, k]

    inv_sqrt_c = np.float32(1.0 / np.sqrt(C))
    linmat = np.concatenate(
        [np.asarray(inputs["lin_w0"]) * inv_sqrt_c,
         np.asarray(inputs["lin_w1"]) * inv_sqrt_c], axis=1).astype(np.float32)

    identm = np.eye(C, dtype=np.float32)
    return wmat, bmat, linmat, identm


def kernel(**inputs):
    key = "prog"
    if key not in _cache:
        nc = _build_program()
        orig = nc.to_json_bytes
        nc.to_json_bytes = lambda: _legalize_sync_waits(orig())
        _cache[key] = nc
    nc = _cache[key]

    from concourse.bass_utils import run_bass_kernel_spmd

    wmat, bmat, linmat, identm = _prep_shared(inputs)
    nf = np.asarray(inputs["node_feats"], np.float32)   # [N, C, 9]
    na = np.asarray(inputs["node_attrs"], np.float32)   # [N, E]
    sc = np.asarray(inputs["sc"], np.float32)           # [N, 4*C]

    in_maps = []
    for s in range(NCORES):
        sl = slice(s * NB, (s + 1) * NB)
        xts = np.ascontiguousarray(
            nf[sl].transpose(1, 0, 2).reshape(C, NB * NIRR))
        yts = np.ascontiguousarray(na[sl].T)
        sct0 = np.ascontiguousarray(sc[sl, 0:C].T)
        sct1 = np.ascontiguousarray(
            sc[sl, C:].reshape(NB, C, 3).transpose(1, 0, 2).reshape(C, 3 * NB))
        in_maps.append({
            "xt": xts, "yt": yts, "wmat": wmat, "bmat": bmat,
            "linmat": linmat, "sct0": sct0, "sct1": sct1, "ident": identm,
        })

    res = run_bass_kernel_spmd(nc, in_maps, core_ids=list(range(NCORES)))

    out = np.empty((N, 4 * C), np.float32)
    for s in range(NCORES):
        sl = slice(s * NB, (s + 1) * NB)
        op = res.results[s]["outp"]                     # [C, 4*NB]
        out[sl, 0:C] = op[:, 0:NB].T
        out[sl, C:] = op[:, NB:4 * NB].reshape(
            C, NB, 3).transpose(1, 0, 2).reshape(NB, 3 * C)
    return out


# revision 19
# speedup vs baseline: 1.4825x; 1.4825x over previous
# Trainium2 Bass kernel for EquivariantProductBasisBlock (MACE-style product basis).
#
# Math (per node b, channel c, both output irreps l0 (d=1) / l1 (d=3)):
#   W_nu[k, c]   = sum_e y[b,e] w_nu[e,k,c]              (per-node path weights)
#   F[f, c]      = [x[c,i]*W3[k,c] (36) | W2[k,c] (3) | W1[k,c] (2)]  x2 irreps = 82
#   Y1[c, m]     = sum_f F[f,c] B[f,m]                   (one K=82 matmul, m=360)
#   E[c, m]      = Y1 * (x_p x_q | x_p broadcast)        (elementwise)
#   out[j, D]    = sum_c lin[c,j] * sum_m E[c, (D,m')]   (matmul with colliding out AP
#                                                         -> PSUM accumulates the m'-sum)
# B packs u3/u2/u1 contracted into a single [82, 360] matrix (host-side, tiny).
#
# Sharding: data-parallel over nodes, 256 nodes per core, 8 cores. U/w/lin replicated.

import numpy as np

N, C, NIRR, E = 2048, 128, 9, 10
K3, K2, K1 = 4, 3, 2
NCORES = 8
NB = N // NCORES          # nodes per core (256)
NF = 41                   # features per irrep
NFT = 2 * NF              # 82 total feature rows
MW = 216                  # 4 D-blocks x 54 (45 sym-pq cols + 9 p-cols)
MPAD = 256                # stage-1 matmul N (zero-padded; f32r needs N>=256)
SW = 54                   # per-D width: 45 cyclic-pair cols + 9 t1 cols
GRP = 8                   # nodes per inner group
NGRP = NB // GRP

import os
USE_COLLISION = os.environ.get("K_COLLISION", "1") == "1"
TSPLIT = int(os.environ.get("K_TSPLIT", "184"))   # nodes < TSPLIT: PE collision; rest: DVE reduce

_cache = {}


def _legalize_sync_waits(json_bytes):
    """This toolchain's walrus accepts at most ONE sync wait per instruction.
    Split extra waits onto same-engine Drain instructions inserted before."""
    import json as _json
    j = _json.loads(json_bytes)
    nid = [0]
    for f in j["functions"]:
        for blk in f["blocks"]:
            out = []
            for inst in blk["instructions"]:
                si = inst.get("sync_info") or {}
                waits = si.get("on_wait") or []
                upds = si.get("on_update") or []
                assert len(upds) <= 1, f"{inst['name']}: {len(upds)} updates"
                if len(waits) > 1:
                    for w in waits[:-1]:
                        nid[0] += 1
                        out.append({
                            "debug": inst.get("debug", 0),
                            "engine": inst["engine"],
                            "ins": [], "outs": [],
                            "name": f"LW-{nid[0]}",
                            "opcode": "Drain",
                            "sync_info": {"on_update": [], "on_wait": [w]},
                        })
                    si["on_wait"] = [waits[-1]]
                out.append(inst)
            blk["instructions"] = out
    return _json.dumps(j).encode()


def _build_program():
    import concourse.bass as bass
    import concourse.mybir as mybir
    from concourse.tile import TileContext

    fp32 = mybir.dt.float32
    f32r = mybir.dt.float32r
    bf16 = mybir.dt.bfloat16
    nc = bass.Bass()

    xt = nc.dram_tensor("xt", [C, NB * NIRR], fp32, kind="ExternalInput")
    yt = nc.dram_tensor("yt", [E, NB], fp32, kind="ExternalInput")
    wmat = nc.dram_tensor("wmat", [E, 18 * C], fp32, kind="ExternalInput")
    bmat = nc.dram_tensor("bmat", [NFT, MPAD], fp32, kind="ExternalInput")
    linmat = nc.dram_tensor("linmat", [C, 2 * C], fp32, kind="ExternalInput")
    sct0 = nc.dram_tensor("sct0", [C, NB], fp32, kind="ExternalInput")
    sct1 = nc.dram_tensor("sct1", [C, 3 * NB], fp32, kind="ExternalInput")
    ident = nc.dram_tensor("ident", [C, C], fp32, kind="ExternalInput")
    outp = nc.dram_tensor("outp", [C, 4 * NB], fp32, kind="ExternalOutput")

    mult = mybir.AluOpType.mult
    add = mybir.AluOpType.add

    with TileContext(nc) as tc:
        with (
            tc.tile_pool(name="singles", bufs=1) as singles,
            tc.tile_pool(name="px", bufs=6) as px,
            tc.tile_pool(name="pxs", bufs=4) as pxs,
            tc.tile_pool(name="pxx", bufs=4) as pxx,
            tc.tile_pool(name="pxsts", bufs=3) as pxsts,
            tc.tile_pool(name="pe", bufs=10) as pe_pool,
            tc.tile_pool(name="psA", bufs=3, space="PSUM") as psA,      # y1 + setup mms
            tc.tile_pool(name="psT", bufs=2, space="PSUM") as psT,      # transposes
            tc.tile_pool(name="psO", bufs=1, space="PSUM") as psO,      # output accum
        ):
            # ---- setup: load constants ----
            identsb = singles.tile([C, C], f32r, tag="ident")
            nc.gpsimd.dma_start(identsb, ident[:, :])
            bsb = singles.tile([NFT, MPAD], f32r, tag="bmat")
            nc.gpsimd.dma_start(bsb, bmat[:, :])
            linsb = singles.tile([C, 2 * C], fp32, tag="linmat")
            nc.gpsimd.dma_start(linsb, linmat[:, :])
            sc0sb = singles.tile([C, NB], fp32, tag="sct0")
            nc.gpsimd.dma_start(sc0sb, sct0[:, :])
            sc1sb = singles.tile([C, 3 * NB], fp32, tag="sct1")
            nc.gpsimd.dma_start(sc1sb, sct1[:, :])
            wsb = singles.tile([E, 18 * C], f32r, tag="wmat")
            nc.gpsimd.dma_start(wsb, wmat[:, :])
            ytsb = singles.tile([E, NB], f32r, tag="yt")
            nc.gpsimd.dma_start(ytsb, yt[:, :])

            # ---- per-node path weights: W_nu[k,c] for all nodes, both irreps ----
            # wtiles[l][nu] laid out [C, k*NB + b]
            nk = [K3, K2, K1]
            wtiles = [[None] * 3 for _ in range(2)]
            si = 0
            for l in range(2):
                for nu in range(3):
                    t = singles.tile([C, nk[nu] * NB], fp32, tag=f"w_{l}_{nu}")
                    wtiles[l][nu] = t
                    for k in range(nk[nu]):
                        ps = psA.tile([C, 512], fp32, tag="y1")
                        nc.tensor.matmul(
                            ps[:, 0:NB],
                            lhsT=wsb[:, si * C:(si + 1) * C],
                            rhs=ytsb[:, :],
                        )
                        if si % 2 == 1:
                            nc.scalar.copy(t[:, k * NB:(k + 1) * NB], ps[:, 0:NB])
                        else:
                            nc.vector.tensor_copy(
                                t[:, k * NB:(k + 1) * NB], ps[:, 0:NB])
                        si += 1

            # persistent output accumulators (PSUM)
            o0ps = psO.tile([C, 512], fp32, tag="o0")
            o1psa = psO.tile([C, 512], fp32, tag="o1a")
            o1psb = psO.tile([C, 512], fp32, tag="o1b")

            tsplit = 0 if not USE_COLLISION else TSPLIT
            fsb = None
            if tsplit < NB:
                fsb = singles.tile([C, 4 * NB], fp32, tag="fsb")
                lin32 = singles.tile([C, 2 * C], fp32, tag="lin32")
                nc.gpsimd.dma_start(lin32, linmat[:, :])

            # ---- main loop over groups of 8 nodes ----
            for g in range(NGRP):
                x8 = px.tile([C, GRP * NIRR], fp32, tag="x8")
                nc.sync.dma_start(x8, xt[:, g * GRP * NIRR:(g + 1) * GRP * NIRR])
                x8v = x8.rearrange("p (n i) -> p n i", i=NIRR)

                # features Xs: [C, n, 82]
                xs8 = pxs.tile([C, GRP * NFT], f32r, tag="xs8")
                xsv = xs8.rearrange("p (n f) -> p n f", f=NFT)
                for l in range(2):
                    w3v = wtiles[l][0].rearrange("p (k b) -> p b k", b=NB)
                    w3s = w3v[:, g * GRP:(g + 1) * GRP, :]
                    nc.gpsimd.tensor_tensor(
                        out=xsv[:, :, NF * l:NF * l + 36].rearrange(
                            "p n (k i) -> p n k i", i=NIRR),
                        in0=x8v.unsqueeze(2).to_broadcast([C, GRP, K3, NIRR]),
                        in1=w3s.unsqueeze(3).to_broadcast([C, GRP, K3, NIRR]),
                        op=mult,
                    )
                    w2v = wtiles[l][1].rearrange("p (k b) -> p b k", b=NB)
                    nc.gpsimd.tensor_copy(
                        xsv[:, :, NF * l + 36:NF * l + 39],
                        w2v[:, g * GRP:(g + 1) * GRP, :],
                    )
                    w1v = wtiles[l][2].rearrange("p (k b) -> p b k", b=NB)
                    nc.gpsimd.tensor_copy(
                        xsv[:, :, NF * l + 39:NF * l + 41],
                        w1v[:, g * GRP:(g + 1) * GRP, :],
                    )

                # XXsym: [C, n, 54]; col v*9+u = x_u * x_{(u+v)%9} (v=0..4),
                # cols 45:54 = x_p (for the t1 part)
                xx8 = pxx.tile([C, GRP * SW], fp32, tag="xx8")
                xxv = xx8.rearrange("p (n s) -> p n s", s=SW)
                nc.gpsimd.tensor_tensor(
                    out=xxv[:, :, 0:NIRR], in0=x8v, in1=x8v, op=mult)
                for v in range(1, 5):
                    nc.gpsimd.tensor_tensor(
                        out=xxv[:, :, 9 * v:9 * v + 9 - v],
                        in0=x8v[:, :, 0:9 - v], in1=x8v[:, :, v:9], op=mult)
                    nc.gpsimd.tensor_tensor(
                        out=xxv[:, :, 9 * v + 9 - v:9 * v + 9],
                        in0=x8v[:, :, 9 - v:9], in1=x8v[:, :, 0:v], op=mult)
                nc.gpsimd.tensor_copy(xxv[:, :, 45:54], x8v)

                # transpose features, 4 nodes per PSUM bank
                for h in range(2):
                    tps = psT.tile([NFT, 512], f32r, tag="xsT")
                    for j in range(4):
                        nc.tensor.transpose(
                            tps[:, 128 * j:128 * (j + 1)],
                            xsv[:, 4 * h + j, :],
                            identsb[:, :],
                        )
                    tsb = pxsts.tile([NFT, 512], f32r, tag="xsTs")
                    nc.scalar.copy(tsb[:, :], tps[:, :])

                    esbs = []
                    for j in range(4):
                        node = g * GRP + 4 * h + j
                        # stage 1: Y1 = F.T @ B  -> [C, 216(+pad)]
                        y1 = psA.tile([C, 512], fp32, tag="y1")
                        nc.tensor.matmul(
                            y1[:, 0:MPAD],
                            lhsT=tsb[:, 128 * j:128 * (j + 1)],
                            rhs=bsb[:, :],
                        )
                        # E = Y1 * XXsym-broadcast  [C, 4, 54]
                        esb = pe_pool.tile([C, MW], fp32, tag="esb")
                        nc.vector.tensor_tensor(
                            out=esb.rearrange("p (d s) -> p d s", s=SW),
                            in0=y1[:, 0:MW].rearrange("p (d s) -> p d s", s=SW),
                            in1=xxv[:, 4 * h + j, :].unsqueeze(1).to_broadcast(
                                [C, 4, SW]),
                            op=mult,
                        )
                        esbs.append((node, esb))
                    for node, esb in esbs:
                        if node >= tsplit:
                            nc.vector.tensor_reduce(
                                out=fsb[:, node::NB],
                                in_=esb.rearrange("p (d s) -> p d s", s=SW),
                                axis=mybir.AxisListType.X,
                                op=add,
                            )
                    for node, esb in esbs:
                        if node < tsplit:
                            nc.tensor.matmul(
                                o0ps[:, node:node + 1].to_broadcast([C, SW]),
                                lhsT=linsb[:, 0:C],
                                rhs=esb[:, 0:SW],
                            )
                    for node, esb in esbs:
                        if node < tsplit:
                            op1 = o1psa if node < 128 else o1psb
                            nb3 = 3 * (node % 128)
                            nc.tensor.matmul(
                                op1[:, nb3:nb3 + 3].unsqueeze(1).to_broadcast(
                                    [C, SW, 3]),
                                lhsT=linsb[:, C:2 * C],
                                rhs=esb[:, SW:MW].rearrange(
                                    "p (d s) -> p s d", s=SW),
                            )

            if tsplit < NB:
                # tail matmuls for nodes >= tsplit: O = lin.T @ F
                nc.tensor.matmul(
                    o0ps[:, tsplit:NB], lhsT=lin32[:, 0:C],
                    rhs=fsb[:, tsplit:NB])
                f1v = fsb.rearrange("p (d b) -> p b d", d=4)[:, :, 1:4]
                if tsplit < 128:
                    nc.tensor.matmul(
                        o1psa[:, 3 * tsplit:384], lhsT=lin32[:, C:2 * C],
                        rhs=f1v[:, tsplit:128, :])
                lo = max(tsplit, 128)
                nc.tensor.matmul(
                    o1psb[:, 3 * (lo - 128):384], lhsT=lin32[:, C:2 * C],
                    rhs=f1v[:, lo:256, :])

            # ---- add sc, store ----
            outsb = singles.tile([C, 4 * NB], fp32, tag="outsb")
            nc.vector.tensor_tensor(
                out=outsb[:, 0:NB], in0=o0ps[:, 0:NB], in1=sc0sb[:, :], op=add)
            nc.vector.tensor_tensor(
                out=outsb[:, NB:NB + 384], in0=o1psa[:, 0:384],
                in1=sc1sb[:, 0:384], op=add)
            nc.vector.tensor_tensor(
                out=outsb[:, NB + 384:4 * NB], in0=o1psb[:, 0:384],
                in1=sc1sb[:, 384:768], op=add)
            nc.sync.dma_start(outp[:, :], outsb[:, :])

    return nc


def _prep_shared(inputs):
    """Host-side tiny tensors, replicated across cores."""
    u3 = [inputs["u3_l0"], inputs["u3_l1"]]
    u2 = [inputs["u2_l0"], inputs["u2_l1"]]
    u1 = [inputs["u1_l0"], inputs["u1_l1"]]
    w3 = [inputs["w3_l0"], inputs["w3_l1"]]
    w2 = [inputs["w2_l0"], inputs["w2_l1"]]
    w1 = [inputs["w1_l0"], inputs["w1_l1"]]

    # wmat [E, 18*C]: per l: w3 k0..3, w2 k0..2, w1 k0..1, each [E, C]
    cols = []
    for l in range(2):
        for wt, nk in ((w3, K3), (w2, K2), (w1, K1)):
            for k in range(nk):
                cols.append(np.asarray(wt[l][:, k, :]))
    wmat = np.concatenate(cols, axis=1).astype(np.float32)

    # bmat [82, 256]; cols: D in {l0d0, l1d0..2} x 54, then zero pad to 256.
    # Within D: col v*9+u (v=0..4) = symmetrized (p,q) pair (u, (u+v)%9);
    # cols 45:54 = t1 cols (p).  Symmetrization: coef[p,q]+coef[q,p] (p!=q).
    bmat = np.zeros((NFT, MPAD), np.float32)
    dmap = [(0, 0), (1, 0), (1, 1), (1, 2)]
    for D, (l, d) in enumerate(dmap):
        r0 = NF * l
        u3l = np.asarray(u3[l], np.float64)  # [d, 9(p), 9(q), 9(i), K3]
        u2l = np.asarray(u2[l], np.float64)  # [d, 9(p), 9(i=q), K2]
        u1l = np.asarray(u1[l], np.float64)  # [d, 9(p), K1]
        # full coefficient matrix [f=82?41-block, 9, 9] for this D
        coef = np.zeros((NFT, NIRR, NIRR))
        for k in range(K3):
            for i in range(NIRR):
                coef[r0 + k * NIRR + i] = u3l[d, :, :, i, k]
        for k in range(K2):
            coef[r0 + 36 + k] = u2l[d, :, :, k]
        sym = coef + np.transpose(coef, (0, 2, 1))
        for v in range(5):
            for u in range(NIRR):
                q = (u + v) % NIRR
                if v == 0:
                    bmat[:, SW * D + v * 9 + u] = coef[:, u, u]
                else:
                    bmat[:, SW * D + v * 9 + u] = sym[:, u, q]
        for k in range(K1):
            bmat[r0 + 39 + k, SW * D + 45:SW * D + 54] = u1l[d, :, k]

    import ml_dtypes
    inv_sqrt_c = np.float32(1.0 / np.sqrt(C))
    linmat = np.concatenate(
        [np.asarray(inputs["lin_w0"]) * inv_sqrt_c,
         np.asarray(inputs["lin_w1"]) * inv_sqrt_c],
        axis=1).astype(np.float32)

    identm = np.eye(C, dtype=np.float32)
    return wmat, bmat, linmat, identm


def kernel(**inputs):
    key = "prog"
    if key not in _cache:
        nc = _build_program()
        orig = nc.to_json_bytes
        nc.to_json_bytes = lambda: _legalize_sync_waits(orig())
        _cache[key] = nc
    nc = _cache[key]

    from concourse.bass_utils import run_bass_kernel_spmd

    wmat, bmat, linmat, identm = _prep_shared(inputs)
    nf = np.asarray(inputs["node_feats"], np.float32)   # [N, C, 9]
    na = np.asarray(inputs["node_attrs"], np.float32)   # [N, E]
    sc = np.asarray(inputs["sc"], np.float32)           # [N, 4*C]

    in_maps = []
    for s in range(NCORES):
        sl = slice(s * NB, (s + 1) * NB)
        xts = np.ascontiguousarray(
            nf[sl].transpose(1, 0, 2).reshape(C, NB * NIRR))
        yts = np.ascontiguousarray(na[sl].T)
        sct0 = np.ascontiguousarray(sc[sl, 0:C].T)
        sct1 = np.ascontiguousarray(
            sc[sl, C:].reshape(NB, C, 3).transpose(1, 0, 2).reshape(C, 3 * NB))
        in_maps.append({
            "xt": xts, "yt": yts, "wmat": wmat, "bmat": bmat,
            "linmat": linmat, "sct0": sct0, "sct1": sct1, "ident": identm,
        })

    res = run_bass_kernel_spmd(nc, in_maps, core_ids=list(range(NCORES)))

    out = np.empty((N, 4 * C), np.float32)
    for s in range(NCORES):
        sl = slice(s * NB, (s + 1) * NB)
        op = res.results[s]["outp"]                     # [C, 4*NB]
        out[sl, 0:C] = op[:, 0:NB].T
        out[sl, C:] = op[:, NB:4 * NB].reshape(
            C, NB, 3).transpose(1, 0, 2).reshape(NB, 3 * C)
    return out
